# revision 11
# baseline (speedup 1.0000x reference)
"""BilateralSliceApply kernel for 8 Trainium2 NeuronCores.

Math (from the reference):
  out = a * (x0 + x1 + x2) + b, where (a, b) are the 2 channels of the
  bilateral grid trilinearly sliced at (ix(w), iy(h), iz(guide)).

  iz = (guide + 1) * 0.5 * (gd - 1) = 3.5 * guide + 3.5 in [3.5, 7) since
  guide is in [0, 1). On that range the z interpolation is a piecewise
  linear function of iz with knots at 4, 5, 6, expressible in hinge form:

    coeff(g) = base + g * E3 + relu(g - 1/7) * E4
                    + relu(g - 3/7) * E5 + relu(g - 5/7) * E6

  where, with Gz[k] the xy-interpolated grid at z-plane k and
  D[k] = Gz[k+1] - Gz[k]:
    base = Gz[3] + 0.5 D[3],  E3 = 3.5 D[3],  Ek = 3.5 (D[k] - D[k-1]).

  The xy bilinear interpolation is separable: the y direction (per output
  row) is folded into small host-precomputed row tables; the x direction
  is a K=16 matmul against a hat-function matrix on the tensor engine,
  producing 5 plane-pairs (basis x {a,b} side by side) per 128-row block.

Engine split per [128 x 1024] column block (all SBUF data fp16; GPSIMD
cannot touch PSUM on TRN2, so every plane is ACT-evacuated first):
  PE   : 10 plane matmuls into PSUM (f32), 2 per basis pair
  ACT  : evacuates all planes PSUM -> SBUF fp16 (the structural floor)
  Pool : e4's two half-products from SBUF + the SWDGE accumulate DMAs
         that build s = x0+x1+x2
  DVE  : relu basis (4x-mode tensor_scalar), remaining products (2x-mode
         fp16 tensor_tensor), add tree, final out = a*s + b

Sharding: 8 shards = batch (4) x H-halves (2), one per core.
"""

import sys

sys.path.insert(0, "/opt/trn_rl_repo")

from contextlib import ExitStack

import numpy as np

import concourse.bacc as bacc
import concourse.bass as bass
import concourse.mybir as mybir
from concourse import tile
from concourse.bass_utils import run_bass_kernel_spmd

N, C, GH, GW, GD = 4, 2, 16, 16, 8
H, W = 2048, 2048
N_CORES = 8
ROWS_PER_CORE = H // 2          # shard = (batch, h-half)
BLK_R = 128                     # rows per block
N_RBLK = ROWS_PER_CORE // BLK_R  # 8
COL_W = 1024                    # columns per work item
N_CBLK = W // COL_W             # 2
N_PAIRS = 5                     # basis: base, e3, e4, e5, e6 (each [a|b])

F16 = mybir.dt.float16
F32 = mybir.dt.float32

# --- tuning knobs (env-overridable for sweeps) ------------------------------
import os as _os


def _cfg(name, default):
    v = _os.environ.get(name)
    if v is None:
        return default
    if isinstance(default, tuple):
        return tuple(int(x) for x in v.split(",") if x != "")
    return type(default)(v)


# pairs {1:e3, 2:e4, 3:e5, 4:e6} whose phi*plane product runs on Pool
# (from the ACT-evacuated SBUF copy); the rest run on DVE
PROD_POOL = _cfg("K_PROD_POOL", ())
ADD_POOL = _cfg("K_ADD_POOL", 0)        # how many chain adds run on Pool
BASE_PSUM = _cfg("K_BASE_PSUM", 0)      # 1: DVE reads base straight from PSUM
N_ACT_RELU = _cfg("K_ACT_RELU", 0)      # relus on ACT (0..3)
S_MODE = _cfg("K_S_MODE", "swdge")        # "swdge" | "dve"
PSUM_SINGLE = _cfg("K_PSUM_SINGLE", 0)  # 1: single-plane PSUM tiles
PSUM_BUFS = _cfg("K_PSUM_BUFS", 4)
PROD_NARROW = _cfg("K_PROD_NARROW", 1)  # 1: per-half products, narrow phi
POOL_HALF3 = _cfg("K_POOL_HALF3", 1)    # 1: pair 3's a-half product on Pool

V1_ACC = _cfg("K_ACC", 0)   # v1: hinge pairs PE-accumulated onto base pair

# --- scheme v2 (channel-separate planes + optional PE accumulation) -------
SCHEME = _cfg("K_SCHEME", "v15")         # "v1" | "v15" | "v2"
# per-plane product mode, planes ordered (q1a,q1b,q2a,q2b,q3a,q3b,q4a,q4b):
#   d: DVE TT reading the plane straight from PSUM (1x)
#   v: ACT evacuates plane to SBUF f16, DVE TT (2x)
#   g: ACT evacuates, GpSimd TT
#   e: DVE evacuates (1x copy), GpSimd TT
V2_ASSIGN = _cfg("K_V2_ASSIGN", "ddvvgggg")
V2_ACC = _cfg("K_V2_ACC", 2)             # hinges PE-accumulated (q4 down)
V2_RELU = _cfg("K_V2_RELU", "dve")       # "dve" | "act"
V2_FIN = _cfg("K_V2_FIN", "evac")        # "evac" | "psum"

_NC_CACHE = {}


def _build_nc(repeat=1):
    key = (repeat, PROD_POOL, ADD_POOL, BASE_PSUM, N_ACT_RELU, S_MODE,
           PSUM_SINGLE, PSUM_BUFS, PROD_NARROW, POOL_HALF3)
    if key in _NC_CACHE:
        return _NC_CACHE[key]
    nc = bacc.Bacc("TRN2", target_bir_lowering=False, debug=False,
                   enable_asserts=False, num_devices=N_CORES)
    guide = nc.dram_tensor("guide", [ROWS_PER_CORE, W], F16,
                           kind="ExternalInput").ap()
    xin = nc.dram_tensor("xin", [3, ROWS_PER_CORE, W], F16,
                         kind="ExternalInput").ap()
    tabs = nc.dram_tensor("tabs", [N_RBLK, GW, N_PAIRS * 2 * BLK_R], F16,
                          kind="ExternalInput").ap()
    rxt = nc.dram_tensor("rxt", [GW, W], F16, kind="ExternalInput").ap()
    out = nc.dram_tensor("out", [ROWS_PER_CORE, W], F16,
                         kind="ExternalOutput").ap()

    Relu = mybir.ActivationFunctionType.Relu
    addo = mybir.AluOpType.add
    sub = mybir.AluOpType.subtract
    mx = mybir.AluOpType.max
    CK = {2: 1.0 / 7.0, 3: 3.0 / 7.0, 4: 5.0 / 7.0}  # hinge knots in g

    with tile.TileContext(nc) as tc:
        with ExitStack() as ctx:
            const_p = ctx.enter_context(tc.tile_pool(name="const", bufs=1))
            tab_p = ctx.enter_context(tc.tile_pool(name="tab", bufs=2))
            g_p = ctx.enter_context(tc.tile_pool(name="g", bufs=3))
            s_p = ctx.enter_context(tc.tile_pool(name="s", bufs=2))
            r_p = ctx.enter_context(tc.tile_pool(name="r", bufs=2))
            pl_p = ctx.enter_context(tc.tile_pool(name="pl", bufs=3))
            ps_p = ctx.enter_context(tc.tile_pool(
                name="ps", bufs=PSUM_BUFS if PSUM_SINGLE else 2,
                space="PSUM"))
            t_p = ctx.enter_context(tc.tile_pool(name="t", bufs=2))
            acc_p = ctx.enter_context(tc.tile_pool(name="acc", bufs=2))
            out_p = ctx.enter_context(tc.tile_pool(name="o", bufs=3))

            rxt_t = const_p.tile([GW, W], F16)
            nc.sync.dma_start(rxt_t[:], rxt[:])
            relu_bias = {}
            for q in range(2, 5):
                if q - 2 < N_ACT_RELU:
                    bt = const_p.tile([BLK_R, 1], F32, tag=f"bias{q}")
                    nc.vector.memset(bt[:], -CK[q])
                    relu_bias[q] = bt

            for rb in [r for _ in range(repeat) for r in range(N_RBLK)]:
                tab_t = tab_p.tile([GW, N_PAIRS * 2 * BLK_R], F16, tag="tab")
                nc.sync.dma_start(tab_t[:], tabs[rb])
                r0 = rb * BLK_R
                # s = x0 + x1 + x2, one full-width tile per row block so the
                # SWDGE accumulate DMAs are issued 3x per 2 column blocks
                s_t = s_p.tile([BLK_R, W], F16, tag="s")
                if S_MODE == "swdge":
                    nc.gpsimd.dma_start(
                        out=s_t[:], in_=xin[0, r0:r0 + BLK_R, :])
                    for chn in (1, 2):
                        nc.gpsimd.dma_start(
                            out=s_t[:], in_=xin[chn, r0:r0 + BLK_R, :],
                            accum_op=addo)
                else:
                    xt = s_p.tile([BLK_R, 2 * W], F16, tag="x12")
                    nc.sync.dma_start(s_t[:], xin[0, r0:r0 + BLK_R, :])
                    for chn in (1, 2):
                        nc.sync.dma_start(
                            xt[:, (chn - 1) * W:chn * W],
                            xin[chn, r0:r0 + BLK_R, :])
                    nc.vector.tensor_add(s_t[:], s_t[:], xt[:, :W])
                    nc.vector.tensor_add(s_t[:], s_t[:], xt[:, W:])
                for cb in range(N_CBLK):
                    c0 = cb * COL_W
                    # guide. With PROD_NARROW every product op is per-half,
                    # so phi tiles stay narrow; otherwise g is duplicated
                    # into a wide [g|g] tile for one-op two-channel products.
                    gw = 2 if not PROD_NARROW else 1
                    gw_t = g_p.tile([BLK_R, gw * COL_W], F16, tag="g")
                    for half in range(gw):
                        nc.sync.dma_start(
                            gw_t[:, half * COL_W:(half + 1) * COL_W],
                            guide[r0:r0 + BLK_R, c0:c0 + COL_W])

                    # hinge basis r_q = relu(g - ck)
                    rk = {}
                    for q in range(2, 5):
                        r_t = r_p.tile([BLK_R, gw * COL_W], F16, tag=f"r{q}")
                        if q - 2 < N_ACT_RELU:
                            nc.scalar.activation(
                                r_t[:], gw_t[:], Relu, bias=relu_bias[q][:])
                        else:
                            nc.vector.tensor_scalar(
                                r_t[:], gw_t[:], CK[q], 0.0, sub, mx)
                        rk[q] = r_t

                    # PE: plane pairs into PSUM (hinges first, base last so
                    # a PSUM-resident base frees quickly); ACT evacuates.
                    # GPSIMD cannot touch PSUM, so every Pool consumer reads
                    # the evacuated SBUF copy.
                    phi = {1: gw_t, 2: rk[2], 3: rk[3], 4: rk[4]}
                    sb_pair = {}        # q -> evacuated pair (SBUF fp16)
                    base_ps = None      # base kept in PSUM (BASE_PSUM mode)
                    for q in (1, 2, 3, 4, 0):
                        keep_psum = q == 0 and BASE_PSUM
                        dst = None
                        if not keep_psum:
                            dst = pl_p.tile([BLK_R, 2 * COL_W], F16,
                                            tag=f"pl{q}")
                            sb_pair[q] = dst
                        if PSUM_SINGLE:
                            halves = []
                            for half in range(2):
                                p = q * 2 + half
                                ps_t = ps_p.tile(
                                    [BLK_R, COL_W], F32,
                                    tag="psb" if keep_psum else "ps",
                                    bufs=2 if BASE_PSUM else PSUM_BUFS)
                                for mc in range(COL_W // 512):
                                    nc.tensor.matmul(
                                        ps_t[:, mc * 512:(mc + 1) * 512],
                                        tab_t[:, p * BLK_R:(p + 1) * BLK_R],
                                        rxt_t[:,
                                              c0 + mc * 512:c0 + (mc + 1) * 512],
                                        start=True, stop=True)
                                if keep_psum:
                                    halves.append(ps_t)
                                else:
                                    nc.scalar.copy(
                                        dst[:, half * COL_W:
                                            (half + 1) * COL_W], ps_t[:])
                            if keep_psum:
                                base_ps = halves
                        else:
                            ps_t = ps_p.tile(
                                [BLK_R, 2 * COL_W], F32,
                                tag="psb" if keep_psum else "ps",
                                bufs=1 if BASE_PSUM else 2)
                            for half in range(2):
                                p = q * 2 + half
                                po = half * COL_W
                                for mc in range(COL_W // 512):
                                    nc.tensor.matmul(
                                        ps_t[:, po + mc * 512:
                                             po + (mc + 1) * 512],
                                        tab_t[:, p * BLK_R:(p + 1) * BLK_R],
                                        rxt_t[:,
                                              c0 + mc * 512:c0 + (mc + 1) * 512],
                                        start=True, stop=True)
                            if keep_psum:
                                base_ps = [ps_t]
                            else:
                                nc.scalar.copy(dst[:], ps_t[:])

                    # products t_q = phi_q * pair_q from SBUF (Pool or DVE)
                    prods = {}
                    for q in range(1, N_PAIRS):
                        t_t = t_p.tile([BLK_R, 2 * COL_W], F16, tag=f"t{q}")
                        eng = nc.gpsimd if q in PROD_POOL else nc.vector
                        if PROD_NARROW:
                            for half in range(2):
                                heng = eng
                                if (q == 3 and half == 0 and POOL_HALF3
                                        and q not in PROD_POOL):
                                    heng = nc.gpsimd
                                sl = slice(half * COL_W, (half + 1) * COL_W)
                                heng.tensor_mul(t_t[:, sl],
                                                phi[q][:, :COL_W],
                                                sb_pair[q][:, sl])
                        else:
                            eng.tensor_mul(t_t[:], phi[q][:],
                                           sb_pair[q][:])
                        prods[q] = t_t

                    # adds, tree-shaped so Pool and DVE can overlap, and
                    # in-place to save SBUF:
                    # t2 += t3 ; t1 += t4 ; t1 += t2 ; acc = t1+base
                    def _add(eng, out_t, a, b):
                        eng.tensor_add(out_t, a, b)

                    _add(nc.gpsimd if ADD_POOL >= 1 else nc.vector,
                         prods[2][:], prods[2][:], prods[3][:])
                    _add(nc.gpsimd if ADD_POOL >= 2 else nc.vector,
                         prods[1][:], prods[1][:], prods[4][:])
                    w_t = prods[1]
                    _add(nc.gpsimd if ADD_POOL >= 3 else nc.vector,
                         w_t[:], w_t[:], prods[2][:])
                    acc = acc_p.tile([BLK_R, 2 * COL_W], F16, tag="acc")
                    if BASE_PSUM:
                        if len(base_ps) == 2:
                            for half in range(2):
                                sl = slice(half * COL_W, (half + 1) * COL_W)
                                nc.vector.tensor_add(
                                    acc[:, sl], w_t[:, sl],
                                    base_ps[half][:])
                        else:
                            nc.vector.tensor_add(acc[:], w_t[:],
                                                 base_ps[0][:])
                    else:
                        nc.vector.tensor_add(acc[:], w_t[:], sb_pair[0][:])

                    # out = a * s + b
                    o_t = out_p.tile([BLK_R, COL_W], F16, tag="o")
                    nc.vector.tensor_mul(o_t[:], acc[:, :COL_W],
                                         s_t[:, c0:c0 + COL_W])
                    nc.vector.tensor_add(o_t[:], o_t[:], acc[:, COL_W:])
                    nc.sync.dma_start(
                        out[r0:r0 + BLK_R, c0:c0 + COL_W], o_t[:])
    nc.compile()
    _NC_CACHE[key] = nc
    return nc


def _build_nc_v15(repeat=1):
    """v1 pair-wide structure, products inline per pair, and V1_ACC hinge
    pairs PE-accumulated onto the base PSUM pair (identity matmuls)."""
    key = ("v15", repeat, V1_ACC, PROD_POOL, PROD_NARROW, S_MODE)
    if key in _NC_CACHE:
        return _NC_CACHE[key]
    nc = bacc.Bacc("TRN2", target_bir_lowering=False, debug=False,
                   enable_asserts=False, num_devices=N_CORES)
    guide = nc.dram_tensor("guide", [ROWS_PER_CORE, W], F16,
                           kind="ExternalInput").ap()
    xin = nc.dram_tensor("xin", [3, ROWS_PER_CORE, W], F16,
                         kind="ExternalInput").ap()
    tabs = nc.dram_tensor("tabs", [N_RBLK, GW, N_PAIRS * 2 * BLK_R], F16,
                          kind="ExternalInput").ap()
    rxt = nc.dram_tensor("rxt", [GW, W], F16, kind="ExternalInput").ap()
    ident = nc.dram_tensor("ident", [BLK_R, BLK_R], F16,
                           kind="ExternalInput").ap()
    out = nc.dram_tensor("out", [ROWS_PER_CORE, W], F16,
                         kind="ExternalOutput").ap()

    addo = mybir.AluOpType.add
    sub = mybir.AluOpType.subtract
    mx = mybir.AluOpType.max
    CK = {2: 1.0 / 7.0, 3: 3.0 / 7.0, 4: 5.0 / 7.0}
    acc_set = [3, 4, 2, 1][:V1_ACC]
    rem = [q for q in (1, 2, 3, 4) if q not in acc_set]
    order = acc_set + rem

    with tile.TileContext(nc) as tc:
        with ExitStack() as ctx:
            const_p = ctx.enter_context(tc.tile_pool(name="const", bufs=1))
            tab_p = ctx.enter_context(tc.tile_pool(name="tab", bufs=2))
            g_p = ctx.enter_context(tc.tile_pool(name="g", bufs=3))
            s_p = ctx.enter_context(tc.tile_pool(name="s", bufs=2))
            r_p = ctx.enter_context(tc.tile_pool(name="r", bufs=2))
            pl_p = ctx.enter_context(tc.tile_pool(name="pl", bufs=3))
            ps_p = ctx.enter_context(tc.tile_pool(name="ps", bufs=2,
                                                  space="PSUM"))
            t_p = ctx.enter_context(tc.tile_pool(name="t", bufs=2))
            acc_p = ctx.enter_context(tc.tile_pool(name="acc", bufs=2))
            out_p = ctx.enter_context(tc.tile_pool(name="o", bufs=3))

            rxt_t = const_p.tile([GW, W], F16)
            nc.sync.dma_start(rxt_t[:], rxt[:])
            ident_t = const_p.tile([BLK_R, BLK_R], F16, tag="id")
            nc.sync.dma_start(ident_t[:], ident[:])

            for rb in [r for _ in range(repeat) for r in range(N_RBLK)]:
                tab_t = tab_p.tile([GW, N_PAIRS * 2 * BLK_R], F16, tag="tab")
                nc.sync.dma_start(tab_t[:], tabs[rb])
                r0 = rb * BLK_R
                s_t = s_p.tile([BLK_R, W], F16, tag="s")
                if S_MODE == "swdge":
                    nc.gpsimd.dma_start(
                        out=s_t[:], in_=xin[0, r0:r0 + BLK_R, :])
                    for chn in (1, 2):
                        nc.gpsimd.dma_start(
                            out=s_t[:], in_=xin[chn, r0:r0 + BLK_R, :],
                            accum_op=addo)
                else:
                    xt = s_p.tile([BLK_R, 2 * W], F16, tag="x12")
                    nc.sync.dma_start(s_t[:], xin[0, r0:r0 + BLK_R, :])
                    for chn in (1, 2):
                        nc.sync.dma_start(
                            xt[:, (chn - 1) * W:chn * W],
                            xin[chn, r0:r0 + BLK_R, :])
                    nc.vector.tensor_add(s_t[:], s_t[:], xt[:, :W])
                    nc.vector.tensor_add(s_t[:], s_t[:], xt[:, W:])
                for cb in range(N_CBLK):
                    c0 = cb * COL_W
                    gw_t = g_p.tile([BLK_R, COL_W], F16, tag="g")
                    nc.sync.dma_start(gw_t[:],
                                      guide[r0:r0 + BLK_R, c0:c0 + COL_W])
                    rk = {}
                    for q in (2, 3, 4):
                        r_t = r_p.tile([BLK_R, COL_W], F16, tag=f"r{q}")
                        nc.vector.tensor_scalar(
                            r_t[:], gw_t[:], CK[q], 0.0, sub, mx)
                        rk[q] = r_t
                    phi = {1: gw_t, 2: rk[2], 3: rk[3], 4: rk[4]}

                    prods = {}
                    for q in order:
                        ps_t = ps_p.tile([BLK_R, 2 * COL_W], F32, tag="ps",
                                         bufs=2)
                        for half in range(2):
                            p = q * 2 + half
                            po = half * COL_W
                            for mc in range(COL_W // 512):
                                nc.tensor.matmul(
                                    ps_t[:, po + mc * 512:po + (mc + 1) * 512],
                                    tab_t[:, p * BLK_R:(p + 1) * BLK_R],
                                    rxt_t[:, c0 + mc * 512:c0 + (mc + 1) * 512],
                                    start=True, stop=True)
                        dst = pl_p.tile([BLK_R, 2 * COL_W], F16, tag=f"pl{q}")
                        nc.scalar.copy(dst[:], ps_t[:])
                        t_t = t_p.tile([BLK_R, 2 * COL_W], F16, tag=f"t{q}")
                        eng = nc.gpsimd if q in PROD_POOL else nc.vector
                        for half in range(2):
                            sl = slice(half * COL_W, (half + 1) * COL_W)
                            eng.tensor_mul(t_t[:, sl], phi[q][:],
                                           dst[:, sl])
                        prods[q] = t_t

                    # base pair, then PE-accumulate acc_set products onto it
                    bs_t = ps_p.tile([BLK_R, 2 * COL_W], F32, tag="ps",
                                     bufs=2)
                    for half in range(2):
                        po = half * COL_W
                        for mc in range(COL_W // 512):
                            nc.tensor.matmul(
                                bs_t[:, po + mc * 512:po + (mc + 1) * 512],
                                tab_t[:, half * BLK_R:(half + 1) * BLK_R],
                                rxt_t[:, c0 + mc * 512:c0 + (mc + 1) * 512],
                                start=True, stop=(V1_ACC == 0))
                    for j, q in enumerate(acc_set):
                        last = j == len(acc_set) - 1
                        for half in range(2):
                            po = half * COL_W
                            for mc in range(COL_W // 512):
                                sl512 = slice(po + mc * 512,
                                              po + (mc + 1) * 512)
                                nc.tensor.matmul(
                                    bs_t[:, sl512], ident_t[:],
                                    prods[q][:, sl512],
                                    start=False, stop=last)
                    base_sb = pl_p.tile([BLK_R, 2 * COL_W], F16, tag="pl0")
                    nc.scalar.copy(base_sb[:], bs_t[:])

                    # DVE: sum remaining hinges + base
                    if rem:
                        cur = prods[rem[0]]
                        for q in rem[1:]:
                            nc.vector.tensor_add(cur[:], cur[:],
                                                 prods[q][:])
                        acc = acc_p.tile([BLK_R, 2 * COL_W], F16, tag="acc")
                        nc.vector.tensor_add(acc[:], cur[:], base_sb[:])
                        a_ap, b_ap = acc[:, :COL_W], acc[:, COL_W:]
                    else:
                        a_ap, b_ap = base_sb[:, :COL_W], base_sb[:, COL_W:]

                    o_t = out_p.tile([BLK_R, COL_W], F16, tag="o")
                    nc.vector.tensor_mul(o_t[:], a_ap,
                                         s_t[:, c0:c0 + COL_W])
                    nc.vector.tensor_add(o_t[:], o_t[:], b_ap)
                    nc.sync.dma_start(
                        out[r0:r0 + BLK_R, c0:c0 + COL_W], o_t[:])
    nc.compile()
    _NC_CACHE[key] = nc
    return nc


def _build_nc_v2(repeat=1):
    key = ("v2", repeat, V2_ASSIGN, V2_ACC, V2_RELU, V2_FIN, S_MODE)
    if key in _NC_CACHE:
        return _NC_CACHE[key]
    nc = bacc.Bacc("TRN2", target_bir_lowering=False, debug=False,
                   enable_asserts=False, num_devices=N_CORES)
    guide = nc.dram_tensor("guide", [ROWS_PER_CORE, W], F16,
                           kind="ExternalInput").ap()
    xin = nc.dram_tensor("xin", [3, ROWS_PER_CORE, W], F16,
                         kind="ExternalInput").ap()
    tabs = nc.dram_tensor("tabs", [N_RBLK, GW, N_PAIRS * 2 * BLK_R], F16,
                          kind="ExternalInput").ap()
    rxt = nc.dram_tensor("rxt", [GW, W], F16, kind="ExternalInput").ap()
    ident = nc.dram_tensor("ident", [BLK_R, BLK_R], F16,
                           kind="ExternalInput").ap()
    out = nc.dram_tensor("out", [ROWS_PER_CORE, W], F16,
                         kind="ExternalOutput").ap()

    Relu = mybir.ActivationFunctionType.Relu
    addo = mybir.AluOpType.add
    sub = mybir.AluOpType.subtract
    mx = mybir.AluOpType.max
    CK = {2: 1.0 / 7.0, 3: 3.0 / 7.0, 4: 5.0 / 7.0}
    CW = 1024                      # column width per work item
    NCB = W // CW
    acc_set = [4, 3, 2, 1][:V2_ACC]
    dve_set = [q for q in (1, 2, 3, 4) if q not in acc_set]

    with tile.TileContext(nc) as tc:
        with ExitStack() as ctx:
            const_p = ctx.enter_context(tc.tile_pool(name="const", bufs=1))
            tab_p = ctx.enter_context(tc.tile_pool(name="tab", bufs=2))
            s_p = ctx.enter_context(tc.tile_pool(name="s", bufs=2))
            g_p = ctx.enter_context(tc.tile_pool(name="g", bufs=2))
            r_p = ctx.enter_context(tc.tile_pool(name="r", bufs=2))
            ev_p = ctx.enter_context(tc.tile_pool(name="ev", bufs=3))
            t_p = ctx.enter_context(tc.tile_pool(name="t", bufs=2))
            w_p = ctx.enter_context(tc.tile_pool(name="w", bufs=2))
            asb_p = ctx.enter_context(tc.tile_pool(name="asb", bufs=2))
            o_p = ctx.enter_context(tc.tile_pool(name="o", bufs=3))
            ps_p = ctx.enter_context(tc.tile_pool(name="ps", bufs=1,
                                                  space="PSUM"))

            rxt_t = const_p.tile([GW, W], F16)
            nc.sync.dma_start(rxt_t[:], rxt[:])
            ident_t = const_p.tile([BLK_R, BLK_R], F16)
            nc.sync.dma_start(ident_t[:], ident[:])
            relu_bias = {}
            if V2_RELU == "act":
                for q in (2, 3, 4):
                    bt = const_p.tile([BLK_R, 1], F32, tag=f"vb{q}")
                    nc.vector.memset(bt[:], -CK[q])
                    relu_bias[q] = bt

            def emit_reduce(st):
                """Accumulate + finalize one item (software-pipelined one
                item behind the front stage)."""
                prods, tab_t, s_t, c0, r0 = st
                acc = ps_p.tile([BLK_R, 2 * CW], F32, tag="acc", bufs=1)
                # base planes first (tab stationaries), then all identity
                # MMs grouped so the stationary only changes once
                for ch in (0, 1):
                    po = ch * CW
                    for m in (0, 1):
                        sl = slice(po + m * 512, po + (m + 1) * 512)
                        nc.tensor.matmul(
                            acc[:, sl],
                            tab_t[:, ch * BLK_R:(ch + 1) * BLK_R],
                            rxt_t[:, c0 + m * 512:c0 + (m + 1) * 512],
                            start=True, stop=(V2_ACC == 0))
                for j, q in enumerate(acc_set):
                    last = j == len(acc_set) - 1
                    for ch in (0, 1):
                        po = ch * CW
                        for m in (0, 1):
                            sl = slice(po + m * 512, po + (m + 1) * 512)
                            nc.tensor.matmul(
                                acc[:, sl], ident_t[:],
                                prods[(q, ch)][:, m * 512:(m + 1) * 512],
                                start=False, stop=last)

                # DVE add tree over non-accumulated hinges
                w = {}
                for ch in (0, 1):
                    cur = None
                    for q in dve_set:
                        if cur is None:
                            cur = prods[(q, ch)]
                            continue
                        nt = w_p.tile([BLK_R, CW], F16, tag=f"w{ch}", bufs=2)
                        nc.vector.tensor_add(nt[:], cur[:],
                                             prods[(q, ch)][:])
                        cur = nt
                    w[ch] = cur

                if V2_FIN == "evac":
                    accsb = asb_p.tile([BLK_R, 2 * CW], F16,
                                       tag="accsb", bufs=2)
                    nc.scalar.copy(accsb[:], acc[:])
                    a_src, b_src = accsb[:, :CW], accsb[:, CW:]
                else:
                    a_src, b_src = acc[:, :CW], acc[:, CW:]

                o_t = o_p.tile([BLK_R, CW], F16, tag="o", bufs=3)
                if w[0] is not None:
                    va = w_p.tile([BLK_R, CW], F16, tag="va", bufs=2)
                    nc.vector.tensor_add(va[:], w[0][:], a_src)
                    nc.vector.tensor_mul(o_t[:], va[:], s_t[:, c0:c0 + CW])
                else:
                    nc.vector.tensor_mul(o_t[:], a_src, s_t[:, c0:c0 + CW])
                o2 = o_p.tile([BLK_R, CW], F16, tag="o2", bufs=3)
                if w[1] is not None:
                    vb = w_p.tile([BLK_R, CW], F16, tag="vb", bufs=2)
                    nc.vector.tensor_add(vb[:], w[1][:], b_src)
                    nc.vector.tensor_add(o2[:], o_t[:], vb[:])
                else:
                    nc.vector.tensor_add(o2[:], o_t[:], b_src)
                nc.sync.dma_start(out[r0:r0 + BLK_R, c0:c0 + CW], o2[:])

            pending = None
            for rb in [r for _ in range(repeat) for r in range(N_RBLK)]:
                tab_t = tab_p.tile([GW, N_PAIRS * 2 * BLK_R], F16, tag="tab")
                nc.sync.dma_start(tab_t[:], tabs[rb])
                r0 = rb * BLK_R
                s_t = s_p.tile([BLK_R, W], F16, tag="s")
                if S_MODE == "swdge":
                    nc.gpsimd.dma_start(
                        out=s_t[:], in_=xin[0, r0:r0 + BLK_R, :])
                    for chn in (1, 2):
                        nc.gpsimd.dma_start(
                            out=s_t[:], in_=xin[chn, r0:r0 + BLK_R, :],
                            accum_op=addo)
                else:
                    xt = s_p.tile([BLK_R, 2 * W], F16, tag="x12")
                    nc.sync.dma_start(s_t[:], xin[0, r0:r0 + BLK_R, :])
                    for chn in (1, 2):
                        nc.sync.dma_start(
                            xt[:, (chn - 1) * W:chn * W],
                            xin[chn, r0:r0 + BLK_R, :])
                    nc.vector.tensor_add(s_t[:], s_t[:], xt[:, :W])
                    nc.vector.tensor_add(s_t[:], s_t[:], xt[:, W:])
                for cb in range(NCB):
                    c0 = cb * CW
                    g_t = g_p.tile([BLK_R, CW], F16, tag="g")
                    nc.sync.dma_start(g_t[:], guide[r0:r0 + BLK_R,
                                                    c0:c0 + CW])
                    rk = {}
                    for q in (2, 3, 4):
                        r_t = r_p.tile([BLK_R, CW], F16, tag=f"r{q}")
                        if V2_RELU == "act":
                            nc.scalar.activation(
                                r_t[:], g_t[:], Relu, bias=relu_bias[q][:])
                        else:
                            nc.vector.tensor_scalar(
                                r_t[:], g_t[:], CK[q], 0.0, sub, mx)
                        rk[q] = r_t
                    phi = {1: g_t, 2: rk[2], 3: rk[3], 4: rk[4]}

                    prods = {}
                    for q in (1, 2, 3, 4):
                        for ch in (0, 1):
                            mode = V2_ASSIGN[(q - 1) * 2 + ch]
                            p = q * 2 + ch
                            pl = ps_p.tile([BLK_R, CW], F32, tag="pl",
                                           bufs=2)
                            for m in (0, 1):
                                nc.tensor.matmul(
                                    pl[:, m * 512:(m + 1) * 512],
                                    tab_t[:, p * BLK_R:(p + 1) * BLK_R],
                                    rxt_t[:, c0 + m * 512:
                                          c0 + (m + 1) * 512],
                                    start=True, stop=True)
                            t_t = t_p.tile([BLK_R, CW], F16,
                                           tag=f"t{q}{ch}", bufs=2)
                            if mode == "d":
                                nc.vector.tensor_mul(t_t[:], phi[q][:],
                                                     pl[:])
                            else:
                                sb = ev_p.tile([BLK_R, CW], F16, tag="ev",
                                               bufs=3)
                                if mode == "e":
                                    nc.vector.tensor_copy(sb[:], pl[:])
                                else:
                                    nc.scalar.copy(sb[:], pl[:])
                                eng = nc.vector if mode == "v" else nc.gpsimd
                                eng.tensor_mul(t_t[:], phi[q][:], sb[:])
                            prods[(q, ch)] = t_t

                    if pending is not None:
                        emit_reduce(pending)
                    pending = (prods, tab_t, s_t, c0, r0)
            if pending is not None:
                emit_reduce(pending)
    nc.compile()
    _NC_CACHE[key] = nc
    return nc


def _build_nc_repeat(repeat):
    if SCHEME == "v2":
        return _build_nc_v2(repeat=repeat)
    if SCHEME == "v15":
        return _build_nc_v15(repeat=repeat)
    return _build_nc(repeat=repeat)


def _host_tables(bilateral_grid):
    """Per-(batch, h-half) row tables [N_RBLK, GW, N_PAIRS*2*BLK_R] (fp16)
    and the shared x-interp hat matrix rxt [GW, W] (fp16).

    Plane order inside a table row is p = basis*2 + channel so that each
    PSUM pair holds [basis_a | basis_b]."""
    g64 = np.asarray(bilateral_grid, dtype=np.float64)  # [N,C,GH,GW,GD]
    h = np.arange(H)
    iy = h / (H - 1) * (GH - 1)
    y0 = np.clip(np.floor(iy).astype(np.int64), 0, GH - 1)
    y1 = np.clip(y0 + 1, 0, GH - 1)
    fy = iy - y0
    # grow[n, c, h, j, z]
    grow = ((1.0 - fy)[None, None, :, None, None] * g64[:, :, y0, :, :]
            + fy[None, None, :, None, None] * g64[:, :, y1, :, :])
    D = grow[..., 1:] - grow[..., :-1]
    base = grow[..., 3] + 0.5 * D[..., 3]
    e3 = 3.5 * D[..., 3]
    e4 = 3.5 * (D[..., 4] - D[..., 3])
    e5 = 3.5 * (D[..., 5] - D[..., 4])
    e6 = 3.5 * (D[..., 6] - D[..., 5])
    # [n, c, basis, h, j] -> planes p = basis*2 + c
    pt = np.stack([base, e3, e4, e5, e6], axis=2)
    pt = pt.transpose(0, 2, 1, 3, 4)                # [n, basis, c, h, j]
    pt = pt.reshape(N, N_PAIRS * 2, H, GW)
    # tabs[n, half, rblk, j, p*r]
    pt = pt.transpose(0, 2, 3, 1)                   # [n, h, j, p]
    pt = pt.reshape(N, 2, N_RBLK, BLK_R, GW, N_PAIRS * 2)
    tabs = pt.transpose(0, 1, 2, 4, 5, 3).reshape(
        N, 2, N_RBLK, GW, N_PAIRS * 2 * BLK_R).astype(np.float16)

    w = np.arange(W)
    ix = w / (W - 1) * (GW - 1)
    x0 = np.clip(np.floor(ix).astype(np.int64), 0, GW - 1)
    x1 = np.clip(x0 + 1, 0, GW - 1)
    fx = ix - x0
    rxt_f = np.zeros((GW, W))
    rxt_f[x0, w] += 1.0 - fx
    np.add.at(rxt_f, (x1, w), fx)
    return tabs, rxt_f.astype(np.float16)


def _make_in_maps(inputs):
    guidemap = np.asarray(inputs["guidemap"]).astype(np.float16)
    full_res_input = np.asarray(inputs["full_res_input"]).astype(np.float16)
    tabs, rxt = _host_tables(inputs["bilateral_grid"])
    in_maps = []
    for core in range(N_CORES):
        n, half = divmod(core, 2)
        r0 = half * ROWS_PER_CORE
        m = {
            "guide": np.ascontiguousarray(guidemap[n, r0:r0 + ROWS_PER_CORE]),
            "xin": np.ascontiguousarray(
                full_res_input[n, :, r0:r0 + ROWS_PER_CORE]),
            "tabs": tabs[n, half],
            "rxt": rxt,
        }
        if SCHEME in ("v2", "v15"):
            m["ident"] = np.eye(BLK_R, dtype=np.float16)
        in_maps.append(m)
    return in_maps


def _postprocess_core_out(out):
    return np.asarray(out).astype(np.float32)


def kernel(bilateral_grid, guidemap, full_res_input):
    in_maps = _make_in_maps({
        "bilateral_grid": bilateral_grid,
        "guidemap": guidemap,
        "full_res_input": full_res_input,
    })
    nc = _build_nc_repeat(1)
    res = run_bass_kernel_spmd(nc, in_maps, list(range(N_CORES)), trace=False)
    out = np.empty((N, 1, H, W), dtype=np.float32)
    for core in range(N_CORES):
        n, half = divmod(core, 2)
        r0 = half * ROWS_PER_CORE
        out[n, 0, r0:r0 + ROWS_PER_CORE] = _postprocess_core_out(
            res.results[core]["out"])
    return out



# revision 12
# speedup vs baseline: 1.1827x; 1.1827x over previous
"""BilateralSliceApply kernel for 8 Trainium2 NeuronCores.

Math (from the reference):
  out = a * (x0 + x1 + x2) + b, where (a, b) are the 2 channels of the
  bilateral grid trilinearly sliced at (ix(w), iy(h), iz(guide)).

  iz = (guide + 1) * 0.5 * (gd - 1) = 3.5 * guide + 3.5 in [3.5, 7) since
  guide is in [0, 1). On that range the z interpolation is a piecewise
  linear function of iz with knots at 4, 5, 6, expressible in hinge form:

    coeff(g) = base + g * E3 + relu(g - 1/7) * E4
                    + relu(g - 3/7) * E5 + relu(g - 5/7) * E6

  where, with Gz[k] the xy-interpolated grid at z-plane k and
  D[k] = Gz[k+1] - Gz[k]:
    base = Gz[3] + 0.5 D[3],  E3 = 3.5 D[3],  Ek = 3.5 (D[k] - D[k-1]).

  The xy bilinear interpolation is separable: the y direction (per output
  row) is folded into small host-precomputed row tables; the x direction
  is a K=16 matmul against a hat-function matrix on the tensor engine,
  producing 5 plane-pairs (basis x {a,b} side by side) per 128-row block.

Engine split per [128 x 1024] column block, default scheme "v15" with
K_ACC=0 (HW-calibrated: DVE fp16 tensor_tensor measures ~4x, ACT
PSUM->SBUF f16 copy ~2x, GpSimd tensor_tensor only ~0.42 efficiency, so
Pool compute is avoided entirely):
  PE   : 10 plane matmuls into PSUM (f32), 2 per basis pair; with K_ACC>0
         also identity-matmul accumulation of hinge products onto the
         base pair (faster single-shot, but costs power under sustained
         load, so the default is 0)
  ACT  : evacuates all 5 pair planes PSUM -> SBUF fp16 (2x mode)
  Pool : only the SWDGE accumulate DMAs that build s = x0+x1+x2
  DVE  : relu basis (4x tensor_scalar), all products (4x fp16
         tensor_tensor, emitted inline per pair), add tree, final
         out = a*s + b

Sharding: 8 shards = batch (4) x H-halves (2), one per core.
"""

import sys

sys.path.insert(0, "/opt/trn_rl_repo")

from contextlib import ExitStack

import numpy as np

import concourse.bacc as bacc
import concourse.bass as bass
import concourse.mybir as mybir
from concourse import tile
from concourse.bass_utils import run_bass_kernel_spmd

N, C, GH, GW, GD = 4, 2, 16, 16, 8
H, W = 2048, 2048
N_CORES = 8
ROWS_PER_CORE = H // 2          # shard = (batch, h-half)
BLK_R = 128                     # rows per block
N_RBLK = ROWS_PER_CORE // BLK_R  # 8
COL_W = 1024                    # columns per work item
N_CBLK = W // COL_W             # 2
N_PAIRS = 5                     # basis: base, e3, e4, e5, e6 (each [a|b])

F16 = mybir.dt.float16
F32 = mybir.dt.float32

# --- tuning knobs (env-overridable for sweeps) ------------------------------
import os as _os


def _cfg(name, default):
    v = _os.environ.get(name)
    if v is None:
        return default
    if isinstance(default, tuple):
        return tuple(int(x) for x in v.split(",") if x != "")
    return type(default)(v)


# pairs {1:e3, 2:e4, 3:e5, 4:e6} whose phi*plane product runs on Pool
# (from the ACT-evacuated SBUF copy); the rest run on DVE
PROD_POOL = _cfg("K_PROD_POOL", ())
ADD_POOL = _cfg("K_ADD_POOL", 0)        # how many chain adds run on Pool
BASE_PSUM = _cfg("K_BASE_PSUM", 0)      # 1: DVE reads base straight from PSUM
N_ACT_RELU = _cfg("K_ACT_RELU", 0)      # relus on ACT (0..3)
S_MODE = _cfg("K_S_MODE", "swdge")        # "swdge" | "dve"
PSUM_SINGLE = _cfg("K_PSUM_SINGLE", 0)  # 1: single-plane PSUM tiles
PSUM_BUFS = _cfg("K_PSUM_BUFS", 4)
PROD_NARROW = _cfg("K_PROD_NARROW", 1)  # 1: per-half products, narrow phi
POOL_HALF3 = _cfg("K_POOL_HALF3", 1)    # 1: pair 3's a-half product on Pool

V1_ACC = _cfg("K_ACC", 0)   # v1: hinge pairs PE-accumulated onto base pair

# --- scheme v2 (channel-separate planes + optional PE accumulation) -------
SCHEME = _cfg("K_SCHEME", "v15")         # "v1" | "v15" | "v2"
# per-plane product mode, planes ordered (q1a,q1b,q2a,q2b,q3a,q3b,q4a,q4b):
#   d: DVE TT reading the plane straight from PSUM (1x)
#   v: ACT evacuates plane to SBUF f16, DVE TT (2x)
#   g: ACT evacuates, GpSimd TT
#   e: DVE evacuates (1x copy), GpSimd TT
V2_ASSIGN = _cfg("K_V2_ASSIGN", "ddvvgggg")
V2_ACC = _cfg("K_V2_ACC", 2)             # hinges PE-accumulated (q4 down)
V2_RELU = _cfg("K_V2_RELU", "dve")       # "dve" | "act"
V2_FIN = _cfg("K_V2_FIN", "evac")        # "evac" | "psum"

_NC_CACHE = {}


def _build_nc(repeat=1):
    key = (repeat, PROD_POOL, ADD_POOL, BASE_PSUM, N_ACT_RELU, S_MODE,
           PSUM_SINGLE, PSUM_BUFS, PROD_NARROW, POOL_HALF3)
    if key in _NC_CACHE:
        return _NC_CACHE[key]
    nc = bacc.Bacc("TRN2", target_bir_lowering=False, debug=False,
                   enable_asserts=False, num_devices=N_CORES)
    guide = nc.dram_tensor("guide", [ROWS_PER_CORE, W], F16,
                           kind="ExternalInput").ap()
    xin = nc.dram_tensor("xin", [3, ROWS_PER_CORE, W], F16,
                         kind="ExternalInput").ap()
    tabs = nc.dram_tensor("tabs", [N_RBLK, GW, N_PAIRS * 2 * BLK_R], F16,
                          kind="ExternalInput").ap()
    rxt = nc.dram_tensor("rxt", [GW, W], F16, kind="ExternalInput").ap()
    out = nc.dram_tensor("out", [ROWS_PER_CORE, W], F16,
                         kind="ExternalOutput").ap()

    Relu = mybir.ActivationFunctionType.Relu
    addo = mybir.AluOpType.add
    sub = mybir.AluOpType.subtract
    mx = mybir.AluOpType.max
    CK = {2: 1.0 / 7.0, 3: 3.0 / 7.0, 4: 5.0 / 7.0}  # hinge knots in g

    with tile.TileContext(nc) as tc:
        with ExitStack() as ctx:
            const_p = ctx.enter_context(tc.tile_pool(name="const", bufs=1))
            tab_p = ctx.enter_context(tc.tile_pool(name="tab", bufs=2))
            g_p = ctx.enter_context(tc.tile_pool(name="g", bufs=3))
            s_p = ctx.enter_context(tc.tile_pool(name="s", bufs=2))
            r_p = ctx.enter_context(tc.tile_pool(name="r", bufs=2))
            pl_p = ctx.enter_context(tc.tile_pool(name="pl", bufs=3))
            ps_p = ctx.enter_context(tc.tile_pool(
                name="ps", bufs=PSUM_BUFS if PSUM_SINGLE else 2,
                space="PSUM"))
            t_p = ctx.enter_context(tc.tile_pool(name="t", bufs=2))
            acc_p = ctx.enter_context(tc.tile_pool(name="acc", bufs=2))
            out_p = ctx.enter_context(tc.tile_pool(name="o", bufs=3))

            rxt_t = const_p.tile([GW, W], F16)
            nc.sync.dma_start(rxt_t[:], rxt[:])
            relu_bias = {}
            for q in range(2, 5):
                if q - 2 < N_ACT_RELU:
                    bt = const_p.tile([BLK_R, 1], F32, tag=f"bias{q}")
                    nc.vector.memset(bt[:], -CK[q])
                    relu_bias[q] = bt

            for rb in [r for _ in range(repeat) for r in range(N_RBLK)]:
                tab_t = tab_p.tile([GW, N_PAIRS * 2 * BLK_R], F16, tag="tab")
                nc.sync.dma_start(tab_t[:], tabs[rb])
                r0 = rb * BLK_R
                # s = x0 + x1 + x2, one full-width tile per row block so the
                # SWDGE accumulate DMAs are issued 3x per 2 column blocks
                s_t = s_p.tile([BLK_R, W], F16, tag="s")
                if S_MODE == "swdge":
                    nc.gpsimd.dma_start(
                        out=s_t[:], in_=xin[0, r0:r0 + BLK_R, :])
                    for chn in (1, 2):
                        nc.gpsimd.dma_start(
                            out=s_t[:], in_=xin[chn, r0:r0 + BLK_R, :],
                            accum_op=addo)
                else:
                    xt = s_p.tile([BLK_R, 2 * W], F16, tag="x12")
                    nc.sync.dma_start(s_t[:], xin[0, r0:r0 + BLK_R, :])
                    for chn in (1, 2):
                        nc.sync.dma_start(
                            xt[:, (chn - 1) * W:chn * W],
                            xin[chn, r0:r0 + BLK_R, :])
                    nc.vector.tensor_add(s_t[:], s_t[:], xt[:, :W])
                    nc.vector.tensor_add(s_t[:], s_t[:], xt[:, W:])
                for cb in range(N_CBLK):
                    c0 = cb * COL_W
                    # guide. With PROD_NARROW every product op is per-half,
                    # so phi tiles stay narrow; otherwise g is duplicated
                    # into a wide [g|g] tile for one-op two-channel products.
                    gw = 2 if not PROD_NARROW else 1
                    gw_t = g_p.tile([BLK_R, gw * COL_W], F16, tag="g")
                    for half in range(gw):
                        nc.sync.dma_start(
                            gw_t[:, half * COL_W:(half + 1) * COL_W],
                            guide[r0:r0 + BLK_R, c0:c0 + COL_W])

                    # hinge basis r_q = relu(g - ck)
                    rk = {}
                    for q in range(2, 5):
                        r_t = r_p.tile([BLK_R, gw * COL_W], F16, tag=f"r{q}")
                        if q - 2 < N_ACT_RELU:
                            nc.scalar.activation(
                                r_t[:], gw_t[:], Relu, bias=relu_bias[q][:])
                        else:
                            nc.vector.tensor_scalar(
                                r_t[:], gw_t[:], CK[q], 0.0, sub, mx)
                        rk[q] = r_t

                    # PE: plane pairs into PSUM (hinges first, base last so
                    # a PSUM-resident base frees quickly); ACT evacuates.
                    # GPSIMD cannot touch PSUM, so every Pool consumer reads
                    # the evacuated SBUF copy.
                    phi = {1: gw_t, 2: rk[2], 3: rk[3], 4: rk[4]}
                    sb_pair = {}        # q -> evacuated pair (SBUF fp16)
                    base_ps = None      # base kept in PSUM (BASE_PSUM mode)
                    for q in (1, 2, 3, 4, 0):
                        keep_psum = q == 0 and BASE_PSUM
                        dst = None
                        if not keep_psum:
                            dst = pl_p.tile([BLK_R, 2 * COL_W], F16,
                                            tag=f"pl{q}")
                            sb_pair[q] = dst
                        if PSUM_SINGLE:
                            halves = []
                            for half in range(2):
                                p = q * 2 + half
                                ps_t = ps_p.tile(
                                    [BLK_R, COL_W], F32,
                                    tag="psb" if keep_psum else "ps",
                                    bufs=2 if BASE_PSUM else PSUM_BUFS)
                                for mc in range(COL_W // 512):
                                    nc.tensor.matmul(
                                        ps_t[:, mc * 512:(mc + 1) * 512],
                                        tab_t[:, p * BLK_R:(p + 1) * BLK_R],
                                        rxt_t[:,
                                              c0 + mc * 512:c0 + (mc + 1) * 512],
                                        start=True, stop=True)
                                if keep_psum:
                                    halves.append(ps_t)
                                else:
                                    nc.scalar.copy(
                                        dst[:, half * COL_W:
                                            (half + 1) * COL_W], ps_t[:])
                            if keep_psum:
                                base_ps = halves
                        else:
                            ps_t = ps_p.tile(
                                [BLK_R, 2 * COL_W], F32,
                                tag="psb" if keep_psum else "ps",
                                bufs=1 if BASE_PSUM else 2)
                            for half in range(2):
                                p = q * 2 + half
                                po = half * COL_W
                                for mc in range(COL_W // 512):
                                    nc.tensor.matmul(
                                        ps_t[:, po + mc * 512:
                                             po + (mc + 1) * 512],
                                        tab_t[:, p * BLK_R:(p + 1) * BLK_R],
                                        rxt_t[:,
                                              c0 + mc * 512:c0 + (mc + 1) * 512],
                                        start=True, stop=True)
                            if keep_psum:
                                base_ps = [ps_t]
                            else:
                                nc.scalar.copy(dst[:], ps_t[:])

                    # products t_q = phi_q * pair_q from SBUF (Pool or DVE)
                    prods = {}
                    for q in range(1, N_PAIRS):
                        t_t = t_p.tile([BLK_R, 2 * COL_W], F16, tag=f"t{q}")
                        eng = nc.gpsimd if q in PROD_POOL else nc.vector
                        if PROD_NARROW:
                            for half in range(2):
                                heng = eng
                                if (q == 3 and half == 0 and POOL_HALF3
                                        and q not in PROD_POOL):
                                    heng = nc.gpsimd
                                sl = slice(half * COL_W, (half + 1) * COL_W)
                                heng.tensor_mul(t_t[:, sl],
                                                phi[q][:, :COL_W],
                                                sb_pair[q][:, sl])
                        else:
                            eng.tensor_mul(t_t[:], phi[q][:],
                                           sb_pair[q][:])
                        prods[q] = t_t

                    # adds, tree-shaped so Pool and DVE can overlap, and
                    # in-place to save SBUF:
                    # t2 += t3 ; t1 += t4 ; t1 += t2 ; acc = t1+base
                    def _add(eng, out_t, a, b):
                        eng.tensor_add(out_t, a, b)

                    _add(nc.gpsimd if ADD_POOL >= 1 else nc.vector,
                         prods[2][:], prods[2][:], prods[3][:])
                    _add(nc.gpsimd if ADD_POOL >= 2 else nc.vector,
                         prods[1][:], prods[1][:], prods[4][:])
                    w_t = prods[1]
                    _add(nc.gpsimd if ADD_POOL >= 3 else nc.vector,
                         w_t[:], w_t[:], prods[2][:])
                    acc = acc_p.tile([BLK_R, 2 * COL_W], F16, tag="acc")
                    if BASE_PSUM:
                        if len(base_ps) == 2:
                            for half in range(2):
                                sl = slice(half * COL_W, (half + 1) * COL_W)
                                nc.vector.tensor_add(
                                    acc[:, sl], w_t[:, sl],
                                    base_ps[half][:])
                        else:
                            nc.vector.tensor_add(acc[:], w_t[:],
                                                 base_ps[0][:])
                    else:
                        nc.vector.tensor_add(acc[:], w_t[:], sb_pair[0][:])

                    # out = a * s + b
                    o_t = out_p.tile([BLK_R, COL_W], F16, tag="o")
                    nc.vector.tensor_mul(o_t[:], acc[:, :COL_W],
                                         s_t[:, c0:c0 + COL_W])
                    nc.vector.tensor_add(o_t[:], o_t[:], acc[:, COL_W:])
                    nc.sync.dma_start(
                        out[r0:r0 + BLK_R, c0:c0 + COL_W], o_t[:])
    nc.compile()
    _NC_CACHE[key] = nc
    return nc


def _build_nc_v15(repeat=1):
    """v1 pair-wide structure, products inline per pair, and V1_ACC hinge
    pairs PE-accumulated onto the base PSUM pair (identity matmuls)."""
    key = ("v15", repeat, V1_ACC, PROD_POOL, PROD_NARROW, S_MODE)
    if key in _NC_CACHE:
        return _NC_CACHE[key]
    nc = bacc.Bacc("TRN2", target_bir_lowering=False, debug=False,
                   enable_asserts=False, num_devices=N_CORES)
    guide = nc.dram_tensor("guide", [ROWS_PER_CORE, W], F16,
                           kind="ExternalInput").ap()
    xin = nc.dram_tensor("xin", [3, ROWS_PER_CORE, W], F16,
                         kind="ExternalInput").ap()
    tabs = nc.dram_tensor("tabs", [N_RBLK, GW, N_PAIRS * 2 * BLK_R], F16,
                          kind="ExternalInput").ap()
    rxt = nc.dram_tensor("rxt", [GW, W], F16, kind="ExternalInput").ap()
    ident = nc.dram_tensor("ident", [BLK_R, BLK_R], F16,
                           kind="ExternalInput").ap()
    out = nc.dram_tensor("out", [ROWS_PER_CORE, W], F16,
                         kind="ExternalOutput").ap()

    addo = mybir.AluOpType.add
    sub = mybir.AluOpType.subtract
    mx = mybir.AluOpType.max
    CK = {2: 1.0 / 7.0, 3: 3.0 / 7.0, 4: 5.0 / 7.0}
    acc_set = [3, 4, 2, 1][:V1_ACC]
    rem = [q for q in (1, 2, 3, 4) if q not in acc_set]
    order = acc_set + rem

    with tile.TileContext(nc) as tc:
        with ExitStack() as ctx:
            const_p = ctx.enter_context(tc.tile_pool(name="const", bufs=1))
            tab_p = ctx.enter_context(tc.tile_pool(name="tab", bufs=2))
            g_p = ctx.enter_context(tc.tile_pool(name="g", bufs=3))
            s_p = ctx.enter_context(tc.tile_pool(name="s", bufs=2))
            r_p = ctx.enter_context(tc.tile_pool(name="r", bufs=2))
            pl_p = ctx.enter_context(tc.tile_pool(name="pl", bufs=3))
            ps_p = ctx.enter_context(tc.tile_pool(name="ps", bufs=2,
                                                  space="PSUM"))
            t_p = ctx.enter_context(tc.tile_pool(name="t", bufs=2))
            acc_p = ctx.enter_context(tc.tile_pool(name="acc", bufs=2))
            out_p = ctx.enter_context(tc.tile_pool(name="o", bufs=3))

            rxt_t = const_p.tile([GW, W], F16)
            nc.sync.dma_start(rxt_t[:], rxt[:])
            ident_t = const_p.tile([BLK_R, BLK_R], F16, tag="id")
            nc.sync.dma_start(ident_t[:], ident[:])

            for rb in [r for _ in range(repeat) for r in range(N_RBLK)]:
                tab_t = tab_p.tile([GW, N_PAIRS * 2 * BLK_R], F16, tag="tab")
                nc.sync.dma_start(tab_t[:], tabs[rb])
                r0 = rb * BLK_R
                s_t = s_p.tile([BLK_R, W], F16, tag="s")
                if S_MODE == "swdge":
                    nc.gpsimd.dma_start(
                        out=s_t[:], in_=xin[0, r0:r0 + BLK_R, :])
                    for chn in (1, 2):
                        nc.gpsimd.dma_start(
                            out=s_t[:], in_=xin[chn, r0:r0 + BLK_R, :],
                            accum_op=addo)
                else:
                    xt = s_p.tile([BLK_R, 2 * W], F16, tag="x12")
                    nc.sync.dma_start(s_t[:], xin[0, r0:r0 + BLK_R, :])
                    for chn in (1, 2):
                        nc.sync.dma_start(
                            xt[:, (chn - 1) * W:chn * W],
                            xin[chn, r0:r0 + BLK_R, :])
                    nc.vector.tensor_add(s_t[:], s_t[:], xt[:, :W])
                    nc.vector.tensor_add(s_t[:], s_t[:], xt[:, W:])
                for cb in range(N_CBLK):
                    c0 = cb * COL_W
                    gw_t = g_p.tile([BLK_R, COL_W], F16, tag="g")
                    nc.sync.dma_start(gw_t[:],
                                      guide[r0:r0 + BLK_R, c0:c0 + COL_W])
                    rk = {}
                    for q in (2, 3, 4):
                        r_t = r_p.tile([BLK_R, COL_W], F16, tag=f"r{q}")
                        nc.vector.tensor_scalar(
                            r_t[:], gw_t[:], CK[q], 0.0, sub, mx)
                        rk[q] = r_t
                    phi = {1: gw_t, 2: rk[2], 3: rk[3], 4: rk[4]}

                    prods = {}
                    for q in order:
                        ps_t = ps_p.tile([BLK_R, 2 * COL_W], F32, tag="ps",
                                         bufs=2)
                        for half in range(2):
                            p = q * 2 + half
                            po = half * COL_W
                            for mc in range(COL_W // 512):
                                nc.tensor.matmul(
                                    ps_t[:, po + mc * 512:po + (mc + 1) * 512],
                                    tab_t[:, p * BLK_R:(p + 1) * BLK_R],
                                    rxt_t[:, c0 + mc * 512:c0 + (mc + 1) * 512],
                                    start=True, stop=True)
                        dst = pl_p.tile([BLK_R, 2 * COL_W], F16, tag=f"pl{q}")
                        nc.scalar.copy(dst[:], ps_t[:])
                        t_t = t_p.tile([BLK_R, 2 * COL_W], F16, tag=f"t{q}")
                        eng = nc.gpsimd if q in PROD_POOL else nc.vector
                        for half in range(2):
                            sl = slice(half * COL_W, (half + 1) * COL_W)
                            eng.tensor_mul(t_t[:, sl], phi[q][:],
                                           dst[:, sl])
                        prods[q] = t_t

                    # base pair, then PE-accumulate acc_set products onto it
                    bs_t = ps_p.tile([BLK_R, 2 * COL_W], F32, tag="ps",
                                     bufs=2)
                    for half in range(2):
                        po = half * COL_W
                        for mc in range(COL_W // 512):
                            nc.tensor.matmul(
                                bs_t[:, po + mc * 512:po + (mc + 1) * 512],
                                tab_t[:, half * BLK_R:(half + 1) * BLK_R],
                                rxt_t[:, c0 + mc * 512:c0 + (mc + 1) * 512],
                                start=True, stop=(V1_ACC == 0))
                    for j, q in enumerate(acc_set):
                        last = j == len(acc_set) - 1
                        for half in range(2):
                            po = half * COL_W
                            for mc in range(COL_W // 512):
                                sl512 = slice(po + mc * 512,
                                              po + (mc + 1) * 512)
                                nc.tensor.matmul(
                                    bs_t[:, sl512], ident_t[:],
                                    prods[q][:, sl512],
                                    start=False, stop=last)
                    base_sb = pl_p.tile([BLK_R, 2 * COL_W], F16, tag="pl0")
                    nc.scalar.copy(base_sb[:], bs_t[:])

                    # DVE: sum remaining hinges + base
                    if rem:
                        cur = prods[rem[0]]
                        for q in rem[1:]:
                            nc.vector.tensor_add(cur[:], cur[:],
                                                 prods[q][:])
                        acc = acc_p.tile([BLK_R, 2 * COL_W], F16, tag="acc")
                        nc.vector.tensor_add(acc[:], cur[:], base_sb[:])
                        a_ap, b_ap = acc[:, :COL_W], acc[:, COL_W:]
                    else:
                        a_ap, b_ap = base_sb[:, :COL_W], base_sb[:, COL_W:]

                    o_t = out_p.tile([BLK_R, COL_W], F16, tag="o")
                    nc.vector.tensor_mul(o_t[:], a_ap,
                                         s_t[:, c0:c0 + COL_W])
                    nc.vector.tensor_add(o_t[:], o_t[:], b_ap)
                    nc.sync.dma_start(
                        out[r0:r0 + BLK_R, c0:c0 + COL_W], o_t[:])
    nc.compile()
    _NC_CACHE[key] = nc
    return nc


def _build_nc_v2(repeat=1):
    key = ("v2", repeat, V2_ASSIGN, V2_ACC, V2_RELU, V2_FIN, S_MODE)
    if key in _NC_CACHE:
        return _NC_CACHE[key]
    nc = bacc.Bacc("TRN2", target_bir_lowering=False, debug=False,
                   enable_asserts=False, num_devices=N_CORES)
    guide = nc.dram_tensor("guide", [ROWS_PER_CORE, W], F16,
                           kind="ExternalInput").ap()
    xin = nc.dram_tensor("xin", [3, ROWS_PER_CORE, W], F16,
                         kind="ExternalInput").ap()
    tabs = nc.dram_tensor("tabs", [N_RBLK, GW, N_PAIRS * 2 * BLK_R], F16,
                          kind="ExternalInput").ap()
    rxt = nc.dram_tensor("rxt", [GW, W], F16, kind="ExternalInput").ap()
    ident = nc.dram_tensor("ident", [BLK_R, BLK_R], F16,
                           kind="ExternalInput").ap()
    out = nc.dram_tensor("out", [ROWS_PER_CORE, W], F16,
                         kind="ExternalOutput").ap()

    Relu = mybir.ActivationFunctionType.Relu
    addo = mybir.AluOpType.add
    sub = mybir.AluOpType.subtract
    mx = mybir.AluOpType.max
    CK = {2: 1.0 / 7.0, 3: 3.0 / 7.0, 4: 5.0 / 7.0}
    CW = 1024                      # column width per work item
    NCB = W // CW
    acc_set = [4, 3, 2, 1][:V2_ACC]
    dve_set = [q for q in (1, 2, 3, 4) if q not in acc_set]

    with tile.TileContext(nc) as tc:
        with ExitStack() as ctx:
            const_p = ctx.enter_context(tc.tile_pool(name="const", bufs=1))
            tab_p = ctx.enter_context(tc.tile_pool(name="tab", bufs=2))
            s_p = ctx.enter_context(tc.tile_pool(name="s", bufs=2))
            g_p = ctx.enter_context(tc.tile_pool(name="g", bufs=2))
            r_p = ctx.enter_context(tc.tile_pool(name="r", bufs=2))
            ev_p = ctx.enter_context(tc.tile_pool(name="ev", bufs=3))
            t_p = ctx.enter_context(tc.tile_pool(name="t", bufs=2))
            w_p = ctx.enter_context(tc.tile_pool(name="w", bufs=2))
            asb_p = ctx.enter_context(tc.tile_pool(name="asb", bufs=2))
            o_p = ctx.enter_context(tc.tile_pool(name="o", bufs=3))
            ps_p = ctx.enter_context(tc.tile_pool(name="ps", bufs=1,
                                                  space="PSUM"))

            rxt_t = const_p.tile([GW, W], F16)
            nc.sync.dma_start(rxt_t[:], rxt[:])
            ident_t = const_p.tile([BLK_R, BLK_R], F16)
            nc.sync.dma_start(ident_t[:], ident[:])
            relu_bias = {}
            if V2_RELU == "act":
                for q in (2, 3, 4):
                    bt = const_p.tile([BLK_R, 1], F32, tag=f"vb{q}")
                    nc.vector.memset(bt[:], -CK[q])
                    relu_bias[q] = bt

            def emit_reduce(st):
                """Accumulate + finalize one item (software-pipelined one
                item behind the front stage)."""
                prods, tab_t, s_t, c0, r0 = st
                acc = ps_p.tile([BLK_R, 2 * CW], F32, tag="acc", bufs=1)
                # base planes first (tab stationaries), then all identity
                # MMs grouped so the stationary only changes once
                for ch in (0, 1):
                    po = ch * CW
                    for m in (0, 1):
                        sl = slice(po + m * 512, po + (m + 1) * 512)
                        nc.tensor.matmul(
                            acc[:, sl],
                            tab_t[:, ch * BLK_R:(ch + 1) * BLK_R],
                            rxt_t[:, c0 + m * 512:c0 + (m + 1) * 512],
                            start=True, stop=(V2_ACC == 0))
                for j, q in enumerate(acc_set):
                    last = j == len(acc_set) - 1
                    for ch in (0, 1):
                        po = ch * CW
                        for m in (0, 1):
                            sl = slice(po + m * 512, po + (m + 1) * 512)
                            nc.tensor.matmul(
                                acc[:, sl], ident_t[:],
                                prods[(q, ch)][:, m * 512:(m + 1) * 512],
                                start=False, stop=last)

                # DVE add tree over non-accumulated hinges
                w = {}
                for ch in (0, 1):
                    cur = None
                    for q in dve_set:
                        if cur is None:
                            cur = prods[(q, ch)]
                            continue
                        nt = w_p.tile([BLK_R, CW], F16, tag=f"w{ch}", bufs=2)
                        nc.vector.tensor_add(nt[:], cur[:],
                                             prods[(q, ch)][:])
                        cur = nt
                    w[ch] = cur

                if V2_FIN == "evac":
                    accsb = asb_p.tile([BLK_R, 2 * CW], F16,
                                       tag="accsb", bufs=2)
                    nc.scalar.copy(accsb[:], acc[:])
                    a_src, b_src = accsb[:, :CW], accsb[:, CW:]
                else:
                    a_src, b_src = acc[:, :CW], acc[:, CW:]

                o_t = o_p.tile([BLK_R, CW], F16, tag="o", bufs=3)
                if w[0] is not None:
                    va = w_p.tile([BLK_R, CW], F16, tag="va", bufs=2)
                    nc.vector.tensor_add(va[:], w[0][:], a_src)
                    nc.vector.tensor_mul(o_t[:], va[:], s_t[:, c0:c0 + CW])
                else:
                    nc.vector.tensor_mul(o_t[:], a_src, s_t[:, c0:c0 + CW])
                o2 = o_p.tile([BLK_R, CW], F16, tag="o2", bufs=3)
                if w[1] is not None:
                    vb = w_p.tile([BLK_R, CW], F16, tag="vb", bufs=2)
                    nc.vector.tensor_add(vb[:], w[1][:], b_src)
                    nc.vector.tensor_add(o2[:], o_t[:], vb[:])
                else:
                    nc.vector.tensor_add(o2[:], o_t[:], b_src)
                nc.sync.dma_start(out[r0:r0 + BLK_R, c0:c0 + CW], o2[:])

            pending = None
            for rb in [r for _ in range(repeat) for r in range(N_RBLK)]:
                tab_t = tab_p.tile([GW, N_PAIRS * 2 * BLK_R], F16, tag="tab")
                nc.sync.dma_start(tab_t[:], tabs[rb])
                r0 = rb * BLK_R
                s_t = s_p.tile([BLK_R, W], F16, tag="s")
                if S_MODE == "swdge":
                    nc.gpsimd.dma_start(
                        out=s_t[:], in_=xin[0, r0:r0 + BLK_R, :])
                    for chn in (1, 2):
                        nc.gpsimd.dma_start(
                            out=s_t[:], in_=xin[chn, r0:r0 + BLK_R, :],
                            accum_op=addo)
                else:
                    xt = s_p.tile([BLK_R, 2 * W], F16, tag="x12")
                    nc.sync.dma_start(s_t[:], xin[0, r0:r0 + BLK_R, :])
                    for chn in (1, 2):
                        nc.sync.dma_start(
                            xt[:, (chn - 1) * W:chn * W],
                            xin[chn, r0:r0 + BLK_R, :])
                    nc.vector.tensor_add(s_t[:], s_t[:], xt[:, :W])
                    nc.vector.tensor_add(s_t[:], s_t[:], xt[:, W:])
                for cb in range(NCB):
                    c0 = cb * CW
                    g_t = g_p.tile([BLK_R, CW], F16, tag="g")
                    nc.sync.dma_start(g_t[:], guide[r0:r0 + BLK_R,
                                                    c0:c0 + CW])
                    rk = {}
                    for q in (2, 3, 4):
                        r_t = r_p.tile([BLK_R, CW], F16, tag=f"r{q}")
                        if V2_RELU == "act":
                            nc.scalar.activation(
                                r_t[:], g_t[:], Relu, bias=relu_bias[q][:])
                        else:
                            nc.vector.tensor_scalar(
                                r_t[:], g_t[:], CK[q], 0.0, sub, mx)
                        rk[q] = r_t
                    phi = {1: g_t, 2: rk[2], 3: rk[3], 4: rk[4]}

                    prods = {}
                    for q in (1, 2, 3, 4):
                        for ch in (0, 1):
                            mode = V2_ASSIGN[(q - 1) * 2 + ch]
                            p = q * 2 + ch
                            pl = ps_p.tile([BLK_R, CW], F32, tag="pl",
                                           bufs=2)
                            for m in (0, 1):
                                nc.tensor.matmul(
                                    pl[:, m * 512:(m + 1) * 512],
                                    tab_t[:, p * BLK_R:(p + 1) * BLK_R],
                                    rxt_t[:, c0 + m * 512:
                                          c0 + (m + 1) * 512],
                                    start=True, stop=True)
                            t_t = t_p.tile([BLK_R, CW], F16,
                                           tag=f"t{q}{ch}", bufs=2)
                            if mode == "d":
                                nc.vector.tensor_mul(t_t[:], phi[q][:],
                                                     pl[:])
                            else:
                                sb = ev_p.tile([BLK_R, CW], F16, tag="ev",
                                               bufs=3)
                                if mode == "e":
                                    nc.vector.tensor_copy(sb[:], pl[:])
                                else:
                                    nc.scalar.copy(sb[:], pl[:])
                                eng = nc.vector if mode == "v" else nc.gpsimd
                                eng.tensor_mul(t_t[:], phi[q][:], sb[:])
                            prods[(q, ch)] = t_t

                    if pending is not None:
                        emit_reduce(pending)
                    pending = (prods, tab_t, s_t, c0, r0)
            if pending is not None:
                emit_reduce(pending)
    nc.compile()
    _NC_CACHE[key] = nc
    return nc


def _build_nc_repeat(repeat):
    if SCHEME == "v2":
        return _build_nc_v2(repeat=repeat)
    if SCHEME == "v15":
        return _build_nc_v15(repeat=repeat)
    return _build_nc(repeat=repeat)


def _host_tables(bilateral_grid):
    """Per-(batch, h-half) row tables [N_RBLK, GW, N_PAIRS*2*BLK_R] (fp16)
    and the shared x-interp hat matrix rxt [GW, W] (fp16).

    Plane order inside a table row is p = basis*2 + channel so that each
    PSUM pair holds [basis_a | basis_b]."""
    g64 = np.asarray(bilateral_grid, dtype=np.float64)  # [N,C,GH,GW,GD]
    h = np.arange(H)
    iy = h / (H - 1) * (GH - 1)
    y0 = np.clip(np.floor(iy).astype(np.int64), 0, GH - 1)
    y1 = np.clip(y0 + 1, 0, GH - 1)
    fy = iy - y0
    # grow[n, c, h, j, z]
    grow = ((1.0 - fy)[None, None, :, None, None] * g64[:, :, y0, :, :]
            + fy[None, None, :, None, None] * g64[:, :, y1, :, :])
    D = grow[..., 1:] - grow[..., :-1]
    base = grow[..., 3] + 0.5 * D[..., 3]
    e3 = 3.5 * D[..., 3]
    e4 = 3.5 * (D[..., 4] - D[..., 3])
    e5 = 3.5 * (D[..., 5] - D[..., 4])
    e6 = 3.5 * (D[..., 6] - D[..., 5])
    # [n, c, basis, h, j] -> planes p = basis*2 + c
    pt = np.stack([base, e3, e4, e5, e6], axis=2)
    pt = pt.transpose(0, 2, 1, 3, 4)                # [n, basis, c, h, j]
    pt = pt.reshape(N, N_PAIRS * 2, H, GW)
    # tabs[n, half, rblk, j, p*r]
    pt = pt.transpose(0, 2, 3, 1)                   # [n, h, j, p]
    pt = pt.reshape(N, 2, N_RBLK, BLK_R, GW, N_PAIRS * 2)
    tabs = pt.transpose(0, 1, 2, 4, 5, 3).reshape(
        N, 2, N_RBLK, GW, N_PAIRS * 2 * BLK_R).astype(np.float16)

    w = np.arange(W)
    ix = w / (W - 1) * (GW - 1)
    x0 = np.clip(np.floor(ix).astype(np.int64), 0, GW - 1)
    x1 = np.clip(x0 + 1, 0, GW - 1)
    fx = ix - x0
    rxt_f = np.zeros((GW, W))
    rxt_f[x0, w] += 1.0 - fx
    np.add.at(rxt_f, (x1, w), fx)
    return tabs, rxt_f.astype(np.float16)


def _make_in_maps(inputs):
    guidemap = np.asarray(inputs["guidemap"]).astype(np.float16)
    full_res_input = np.asarray(inputs["full_res_input"]).astype(np.float16)
    tabs, rxt = _host_tables(inputs["bilateral_grid"])
    in_maps = []
    for core in range(N_CORES):
        n, half = divmod(core, 2)
        r0 = half * ROWS_PER_CORE
        m = {
            "guide": np.ascontiguousarray(guidemap[n, r0:r0 + ROWS_PER_CORE]),
            "xin": np.ascontiguousarray(
                full_res_input[n, :, r0:r0 + ROWS_PER_CORE]),
            "tabs": tabs[n, half],
            "rxt": rxt,
        }
        if SCHEME in ("v2", "v15"):
            m["ident"] = np.eye(BLK_R, dtype=np.float16)
        in_maps.append(m)
    return in_maps


def _postprocess_core_out(out):
    return np.asarray(out).astype(np.float32)


def kernel(bilateral_grid, guidemap, full_res_input):
    in_maps = _make_in_maps({
        "bilateral_grid": bilateral_grid,
        "guidemap": guidemap,
        "full_res_input": full_res_input,
    })
    nc = _build_nc_repeat(1)
    res = run_bass_kernel_spmd(nc, in_maps, list(range(N_CORES)), trace=False)
    out = np.empty((N, 1, H, W), dtype=np.float32)
    for core in range(N_CORES):
        n, half = divmod(core, 2)
        r0 = half * ROWS_PER_CORE
        out[n, 0, r0:r0 + ROWS_PER_CORE] = _postprocess_core_out(
            res.results[core]["out"])
    return out

